# revision 1
# baseline (speedup 1.0000x reference)
"""Conv2D 3x3 (stride 1, pad 1) NCHW on 8 TRN2 NeuronCores.

x: (32, 128, 56, 56) f32, weight: (256, 128, 3, 3) OIHW, bias: (256,)
out: (32, 256, 56, 56) f32.

Strategy: data-parallel over batch (4 images per core, weight/bias
replicated). The input is zero-padded to 58x58 on the host, so each padded
image lives in SBUF with C_in=128 on partitions and needs no on-device
border handling. The 3x3 conv is 9 shifted [128x128] @ [128x448] matmuls
accumulated in PSUM (output tile = 8 rows x 56 cols per co-tile), using
float32r operands (full PE rate, ~1.5e-4 rel err). Bias is added on the
vector engine while evacuating PSUM -> SBUF, then DMA to HBM.
"""

import numpy as np

import concourse.tile as tile
from concourse import bacc, mybir
from concourse.bass_utils import run_bass_kernel_spmd

N_CORES = 8
N_BATCH = 32
N_PER_CORE = N_BATCH // N_CORES  # 4
C_IN, C_OUT, H, W = 128, 256, 56, 56
HP, WP = H + 2, W + 2  # 58 (zero-padded on host)
ROWS = 8  # output rows per PSUM tile
N_RTILES = H // ROWS  # 7
NFREE = ROWS * W  # 448 <= 512 (one PSUM bank; f32r full rate needs >= 256)
N_CT = C_OUT // 128  # 2 co-tiles


def build_nc(n_imgs=N_PER_CORE):
    f32 = mybir.dt.float32
    f32r = mybir.dt.float32r
    nc = bacc.Bacc("TRN2", target_bir_lowering=False, debug=False)
    x = nc.dram_tensor("x", [n_imgs, C_IN, HP, WP], f32r, kind="ExternalInput")
    w = nc.dram_tensor("w", [C_IN, 9 * C_OUT], f32r, kind="ExternalInput")
    b = nc.dram_tensor("b", [C_IN, N_CT], f32, kind="ExternalInput")
    out = nc.dram_tensor("out", [n_imgs, C_OUT, H * W], f32, kind="ExternalOutput")

    with tile.TileContext(nc) as tc:
        with tc.tile_pool(name="wpool", bufs=1) as wpool, \
             tc.tile_pool(name="xpool", bufs=2) as xpool, \
             tc.tile_pool(name="opool", bufs=8) as opool, \
             tc.tile_pool(name="pspool", bufs=4, space="PSUM") as pspool:
            # Startup is posting-bound: descriptor generation costs ~0.6us
            # per DMA per sequencer, so interleave posts across the sync and
            # (idle-at-startup) scalar sequencers. Order: the first two
            # image-0 chunks (the first row-tile's inputs), then the nine
            # weight taps, then the rest of image 0.
            w_sb = wpool.tile([C_IN, 9 * C_OUT], f32r)
            b_sb = wpool.tile([C_IN, N_CT], f32)
            xp0 = xpool.tile([C_IN, HP, WP], f32r, tag="xp", name="xp")
            x_chunks = [(a, min(a + ROWS, HP)) for a in range(0, HP, ROWS)]
            nc.sync.dma_start(xp0[:, x_chunks[0][0]:x_chunks[0][1], :],
                              x[0, :, x_chunks[0][0]:x_chunks[0][1], :])
            nc.scalar.dma_start(xp0[:, x_chunks[1][0]:x_chunks[1][1], :],
                                x[0, :, x_chunks[1][0]:x_chunks[1][1], :])
            for tap in range(9):
                sl = slice(tap * C_OUT, (tap + 1) * C_OUT)
                eng = nc.sync if tap % 2 == 0 else nc.scalar
                eng.dma_start(w_sb[:, sl], w[:, sl])
            nc.sync.dma_start(b_sb[:], b[:])
            for ci, (a, e) in enumerate(x_chunks[2:]):
                eng = nc.scalar if ci % 2 == 0 else nc.sync
                eng.dma_start(xp0[:, a:e, :], x[0, :, a:e, :])

            for n in range(n_imgs):
                if n == 0:
                    xp = xp0
                else:
                    xp = xpool.tile([C_IN, HP, WP], f32r, tag="xp", name="xp")
                    # later images prefetch under compute; HWDGE via sync
                    # (SWDGE descriptor traffic slows the PE's SBUF stream)
                    for a, e in x_chunks:
                        nc.sync.dma_start(xp[:, a:e, :], x[n, :, a:e, :])
                for r in range(N_RTILES):
                    for ct in range(N_CT):
                        pt = pspool.tile([128, NFREE], f32, tag="pt")
                        for tap in range(9):
                            kh, kw = tap // 3, tap % 3
                            c0 = tap * C_OUT + ct * 128
                            nc.tensor.matmul(
                                pt[:],
                                w_sb[:, c0:c0 + 128],
                                xp[:, r * ROWS + kh:r * ROWS + kh + ROWS, kw:kw + W],
                                start=(tap == 0),
                                stop=(tap == 8),
                            )
                        ot = opool.tile([128, NFREE], f32, tag="ot")
                        nc.vector.tensor_scalar_add(ot[:], pt[:], b_sb[:, ct:ct + 1])
                        # outputs on sync in halves; the final tile in
                        # quarters so the post-compute drain tail is short
                        last = n == n_imgs - 1 and r == N_RTILES - 1
                        parts = 4 if last else 2
                        step = NFREE // parts
                        for hh in range(parts):
                            nc.sync.dma_start(
                                out[n, ct * 128:(ct + 1) * 128,
                                    r * NFREE + hh * step:r * NFREE + (hh + 1) * step],
                                ot[:, hh * step:(hh + 1) * step],
                            )
    nc.compile()
    return nc


def _host_prep(x, weight, bias):
    # zero-pad H and W by 1 on the host: border handling costs nothing here
    xp = np.pad(np.asarray(x, dtype=np.float32),
                ((0, 0), (0, 0), (1, 1), (1, 1)))
    xp = np.ascontiguousarray(xp)
    # weight OIHW -> [ci, (kh kw co)] so each lhsT tile is a contiguous slice
    w_host = np.ascontiguousarray(
        np.asarray(weight, dtype=np.float32).transpose(1, 2, 3, 0).reshape(C_IN, 9 * C_OUT)
    )
    # bias[co] -> [co % 128, co // 128]
    b_host = np.ascontiguousarray(
        np.asarray(bias, dtype=np.float32).reshape(N_CT, 128).T)
    return xp, w_host, b_host


def kernel(x, weight, bias, _trace=False):
    xp, w_host, b_host = _host_prep(x, weight, bias)
    nc = build_nc()
    in_maps = [
        {"x": xp[i * N_PER_CORE:(i + 1) * N_PER_CORE], "w": w_host, "b": b_host}
        for i in range(N_CORES)
    ]
    res = run_bass_kernel_spmd(nc, in_maps, core_ids=list(range(N_CORES)), trace=_trace)
    out = np.concatenate(
        [res.results[i]["out"].reshape(N_PER_CORE, C_OUT, H, W) for i in range(N_CORES)],
        axis=0,
    )
    if _trace:
        return out, res
    return out



# revision 6
# speedup vs baseline: 1.1022x; 1.1022x over previous
"""Conv2D 3x3 (stride 1, pad 1) NCHW on 8 TRN2 NeuronCores.

x: (32, 128, 56, 56) f32, weight: (256, 128, 3, 3) OIHW, bias: (256,)
out: (32, 256, 56, 56) f32.

Strategy: data-parallel over batch (4 images per core, weight/bias
replicated). The input is zero-padded to 58x58 and cast to bf16 on the
host, so each padded image lives in SBUF with C_in=128 on partitions and
needs no on-device border handling. The 3x3 conv is 9 shifted
[128x128] @ [128x448] bf16 matmuls accumulated in PSUM (output tile =
8 rows x 56 cols per co-tile). bf16 stationary operands let walrus emit
standalone LDWEIGHTS that the PE pulls ahead into the background weight
buffer (hidden under the previous matmul's stream) with FWL engaged —
f32r matmuls are self-loading and expose the weight load serially.
A few dependency-free warmup matmuls run during the startup DMA so the
PE p-state ramp completes before real work arrives. Bias is added on
the vector engine while evacuating PSUM -> SBUF (bf16), then DMA to
HBM; the host upcasts to f32.
"""

import numpy as np
import ml_dtypes

import concourse.tile as tile
from concourse import bacc, mybir
from concourse.bass_utils import run_bass_kernel_spmd

N_CORES = 8
N_BATCH = 32
N_PER_CORE = N_BATCH // N_CORES  # 4
C_IN, C_OUT, H, W = 128, 256, 56, 56
HP, WP = H + 2, W + 2  # 58 (zero-padded on host)
ROWS = 8  # output rows per PSUM tile
N_RTILES = H // ROWS  # 7
NFREE = ROWS * W  # 448 <= 512 (one PSUM bank)
N_CT = C_OUT // 128  # 2 co-tiles
N_WARMUP = 6  # dependency-free matmuls to ride out the PE p-state ramp


def build_nc(n_imgs=N_PER_CORE, repeat=1):
    f32 = mybir.dt.float32
    bf16 = mybir.dt.bfloat16
    nc = bacc.Bacc("TRN2", target_bir_lowering=False, debug=False)
    x = nc.dram_tensor("x", [n_imgs, C_IN, HP, WP], bf16, kind="ExternalInput")
    w = nc.dram_tensor("w", [C_IN, 9 * C_OUT], bf16, kind="ExternalInput")
    b = nc.dram_tensor("b", [C_IN, N_CT], f32, kind="ExternalInput")
    out = nc.dram_tensor("out", [n_imgs, C_OUT, H * W], bf16, kind="ExternalOutput")

    with tile.TileContext(nc) as tc:
        with tc.tile_pool(name="wpool", bufs=1) as wpool, \
             tc.tile_pool(name="xpool", bufs=3) as xpool, \
             tc.tile_pool(name="opool", bufs=8) as opool, \
             tc.tile_pool(name="wmpool", bufs=1, space="PSUM") as wmpool, \
             tc.tile_pool(name="pspool", bufs=6, space="PSUM") as pspool:
            w_sb = wpool.tile([C_IN, 9 * C_OUT], bf16)
            b_sb = wpool.tile([C_IN, N_CT], f32)
            warm = wpool.tile([C_IN, 640], bf16)
            warm_ps = wmpool.tile([128, 512], f32)

            # Startup: HWDGE posting is a shared ~625ns/post resource, so
            # batch into few DMAs. The r=0 tiles need x rows 0:9 and the
            # tap-0..3 weights first; queues run in parallel.
            xp0 = xpool.tile([C_IN, HP, WP], bf16, tag="xp", name="xp")
            nc.sync.dma_start(xp0[:, 0:9, :], x[0, :, 0:9, :])
            nc.scalar.dma_start(w_sb[:, :4 * C_OUT], w[:, :4 * C_OUT])
            nc.sync.dma_start(xp0[:, 9:33, :], x[0, :, 9:33, :])
            nc.scalar.dma_start(w_sb[:, 4 * C_OUT:], w[:, 4 * C_OUT:])
            nc.sync.dma_start(xp0[:, 33:, :], x[0, :, 33:, :])
            nc.scalar.dma_start(b_sb[:], b[:])

            # Warmup: garbage bf16 matmuls keep the PE busy from t~1us so the
            # p-state ramp burns down under the startup DMAs. The warm tile is
            # zeroed on the otherwise-idle gpsimd engine (results discarded).
            nc.gpsimd.memset(warm[:], 0.0)
            for _ in range(N_WARMUP):
                nc.tensor.matmul(warm_ps[:], warm[:, :128], warm[:, 128:640],
                                 start=True, stop=True)

            otile_idx = 0
            for rep in range(repeat):
                for n in range(n_imgs):
                    if rep == 0 and n == 0:
                        xp = xp0
                    else:
                        xp = xpool.tile([C_IN, HP, WP], bf16, tag="xp", name="xp")
                        nc.sync.dma_start(xp[:], x[n, :, :, :])
                    for r in range(N_RTILES):
                        for ct in range(N_CT):
                            # The very last tile accumulates in two half-width
                            # groups so its evacuation overlaps the second
                            # half's matmuls, shortening the drain tail.
                            last = (rep == repeat - 1 and n == n_imgs - 1
                                    and r == N_RTILES - 1 and ct == N_CT - 1)
                            halves = 2 if last else 1
                            hw_ = NFREE // halves   # 448 or 224
                            hrows = ROWS // halves  # 8 or 4 output rows
                            for h in range(halves):
                                r0 = r * ROWS + h * hrows
                                pt = pspool.tile([128, hw_], f32, tag="pt")
                                for tap in range(9):
                                    kh, kw = tap // 3, tap % 3
                                    c0 = tap * C_OUT + ct * 128
                                    nc.tensor.matmul(
                                        pt[:],
                                        w_sb[:, c0:c0 + 128],
                                        xp[:, r0 + kh:r0 + kh + hrows, kw:kw + W],
                                        start=(tap == 0),
                                        stop=(tap == 8),
                                    )
                                ot = opool.tile([128, hw_], bf16, tag="ot")
                                nc.vector.tensor_scalar_add(ot[:], pt[:],
                                                            b_sb[:, ct:ct + 1])
                                eng = nc.sync if otile_idx % 2 == 0 else nc.scalar
                                otile_idx += 1
                                eng.dma_start(
                                    out[n, ct * 128:(ct + 1) * 128,
                                        r * NFREE + h * hw_:r * NFREE + (h + 1) * hw_],
                                    ot[:],
                                )
    nc.compile()
    return nc


def _host_prep(x, weight, bias):
    bf16 = ml_dtypes.bfloat16
    # zero-pad H and W by 1 on the host: border handling costs nothing here
    xp = np.pad(np.asarray(x, dtype=np.float32),
                ((0, 0), (0, 0), (1, 1), (1, 1)))
    xp = np.ascontiguousarray(xp).astype(bf16)
    # weight OIHW -> [ci, (kh kw co)] so each lhsT tile is a contiguous slice
    w_host = np.ascontiguousarray(
        np.asarray(weight, dtype=np.float32).transpose(1, 2, 3, 0).reshape(C_IN, 9 * C_OUT)
    ).astype(bf16)
    # bias[co] -> [co % 128, co // 128]
    b_host = np.ascontiguousarray(
        np.asarray(bias, dtype=np.float32).reshape(N_CT, 128).T)
    return xp, w_host, b_host


def kernel(x, weight, bias, _trace=False, _repeat=1):
    xp, w_host, b_host = _host_prep(x, weight, bias)
    nc = build_nc(repeat=_repeat)
    in_maps = [
        {"x": xp[i * N_PER_CORE:(i + 1) * N_PER_CORE], "w": w_host, "b": b_host}
        for i in range(N_CORES)
    ]
    res = run_bass_kernel_spmd(nc, in_maps, core_ids=list(range(N_CORES)), trace=_trace)
    out = np.concatenate(
        [res.results[i]["out"].astype(np.float32).reshape(N_PER_CORE, C_OUT, H, W)
         for i in range(N_CORES)],
        axis=0,
    )
    if _trace:
        return out, res
    return out


# revision 8
# speedup vs baseline: 1.1394x; 1.0338x over previous
"""Conv2D 3x3 (stride 1, pad 1) NCHW on 8 TRN2 NeuronCores.

x: (32, 128, 56, 56) f32, weight: (256, 128, 3, 3) OIHW, bias: (256,)
out: (32, 256, 56, 56) f32.

Strategy: data-parallel over batch (4 images per core, weight/bias
replicated). The input is zero-padded to 58x58 and cast to bf16 on the
host, so each padded image lives in SBUF with C_in=128 on partitions and
needs no on-device border handling. The 3x3 conv is 9 shifted
[128x128] @ [128x448] bf16 matmuls accumulated in PSUM (output tile =
8 rows x 56 cols per co-tile). bf16 stationary operands let walrus emit
standalone LDWEIGHTS that the PE pulls ahead into the background weight
buffer (hidden under the previous matmul's stream) with FWL engaged —
f32r matmuls are self-loading and expose the weight load serially.
A few dependency-free warmup matmuls run during the startup DMA so the
PE p-state ramp completes before real work arrives. Bias is added on
the vector engine while evacuating PSUM -> SBUF (bf16), then DMA to
HBM; the host upcasts to f32.
"""

import numpy as np
import ml_dtypes

import concourse.tile as tile
from concourse import bacc, mybir
from concourse.bass_utils import run_bass_kernel_spmd

N_CORES = 8
N_BATCH = 32
N_PER_CORE = N_BATCH // N_CORES  # 4
C_IN, C_OUT, H, W = 128, 256, 56, 56
HP, WP = H + 2, W + 2  # 58 (zero-padded on host)
ROWS = 8  # output rows per PSUM tile
N_RTILES = H // ROWS  # 7
NFREE = ROWS * W  # 448 <= 512 (one PSUM bank)
N_CT = C_OUT // 128  # 2 co-tiles
N_WARMUP = 6  # dependency-free matmuls to ride out the PE p-state ramp


def build_nc(n_imgs=N_PER_CORE, repeat=1):
    f32 = mybir.dt.float32
    bf16 = mybir.dt.bfloat16
    nc = bacc.Bacc("TRN2", target_bir_lowering=False, debug=False)
    x = nc.dram_tensor("x", [n_imgs, C_IN, HP, WP], bf16, kind="ExternalInput")
    w = nc.dram_tensor("w", [C_IN, 9 * C_OUT], bf16, kind="ExternalInput")
    b = nc.dram_tensor("b", [C_IN, N_CT], f32, kind="ExternalInput")
    out = nc.dram_tensor("out", [n_imgs, C_OUT, H * W], bf16, kind="ExternalOutput")

    with tile.TileContext(nc) as tc:
        with tc.tile_pool(name="wpool", bufs=1) as wpool, \
             tc.tile_pool(name="xpool", bufs=3) as xpool, \
             tc.tile_pool(name="opool", bufs=8) as opool, \
             tc.tile_pool(name="wmpool", bufs=1, space="PSUM") as wmpool, \
             tc.tile_pool(name="pspool", bufs=6, space="PSUM") as pspool:
            w_sb = wpool.tile([C_IN, 9 * C_OUT], bf16)
            b_sb = wpool.tile([C_IN, N_CT], f32)
            warm = wpool.tile([C_IN, 512], bf16)
            warm_ps = wmpool.tile([128, 512], f32)

            # Startup: HWDGE posting is a shared ~625ns/post resource and the
            # startup transfers contend for HBM, so batch into few DMAs and
            # defer the image 1..3 prefetches into the main loop. The r=0
            # tiles need x rows 0:9 and the weights first.
            xp0 = xpool.tile([C_IN, HP, WP], bf16, tag="xp", name="xp")
            nc.sync.dma_start(xp0[:, 0:9, :], x[0, :, 0:9, :])
            nc.scalar.dma_start(w_sb[:], w[:])
            nc.sync.dma_start(xp0[:, 9:33, :], x[0, :, 9:33, :])
            nc.scalar.dma_start(b_sb[:], b[:])
            nc.sync.dma_start(xp0[:, 33:, :], x[0, :, 33:, :])

            # Warmup: garbage bf16 matmuls keep the PE busy early so the
            # p-state ramp burns down under the startup DMAs. The warm tile
            # is zeroed on the vector engine (results are discarded; the
            # moving operand deliberately overlaps the stationary slice).
            nc.vector.memset(warm[:], 0.0)
            for _ in range(N_WARMUP):
                nc.tensor.matmul(warm_ps[:], warm[:, :128], warm[:],
                                 start=True, stop=True)

            otile_idx = 0
            imgs = [(rep, n) for rep in range(repeat) for n in range(n_imgs)]
            xp = xp0
            for idx, (rep, n) in enumerate(imgs):
                next_xp = None
                for r in range(N_RTILES):
                    for ct in range(N_CT):
                        # Near the end, the output-DMA completion chain is
                        # the critical path: the last row-tiles' outputs go
                        # out as several small DMAs spread over both queues
                        # so the transfers run on parallel DMA engines. The
                        # very last tile also accumulates in two half-width
                        # PSUM groups so its evacuation overlaps the second
                        # half's matmuls.
                        at_end = idx == len(imgs) - 1 and r >= N_RTILES - 2
                        last = (idx == len(imgs) - 1 and r == N_RTILES - 1
                                and ct == N_CT - 1)
                        halves = 2 if last else 1
                        hw_ = NFREE // halves   # 448 or 224
                        hrows = ROWS // halves  # 8 or 4 output rows
                        for h in range(halves):
                            r0 = r * ROWS + h * hrows
                            pt = pspool.tile([128, hw_], f32, tag="pt")
                            for tap in range(9):
                                kh, kw = tap // 3, tap % 3
                                c0 = tap * C_OUT + ct * 128
                                nc.tensor.matmul(
                                    pt[:],
                                    w_sb[:, c0:c0 + 128],
                                    xp[:, r0 + kh:r0 + kh + hrows, kw:kw + W],
                                    start=(tap == 0),
                                    stop=(tap == 8),
                                )
                            ot = opool.tile([128, hw_], bf16, tag="ot")
                            nc.vector.tensor_scalar_add(ot[:], pt[:],
                                                        b_sb[:, ct:ct + 1])
                            pieces = (hw_ // 112) if at_end else 1
                            step = hw_ // pieces
                            for p in range(pieces):
                                eng = nc.sync if otile_idx % 2 == 0 else nc.scalar
                                otile_idx += 1
                                o0 = r * NFREE + h * hw_ + p * step
                                eng.dma_start(
                                    out[n, ct * 128:(ct + 1) * 128, o0:o0 + step],
                                    ot[:, p * step:(p + 1) * step],
                                )
                    # Prefetch the next image once this image's pipeline is
                    # rolling — late enough not to contend with the startup
                    # weight/x transfers, early enough to hide completely.
                    if r == 1 and idx + 1 < len(imgs):
                        n_next = imgs[idx + 1][1]
                        next_xp = xpool.tile([C_IN, HP, WP], bf16,
                                             tag="xp", name="xp")
                        nc.sync.dma_start(next_xp[:], x[n_next, :, :, :])
                if next_xp is not None:
                    xp = next_xp
    nc.compile()
    return nc


def _host_prep(x, weight, bias):
    bf16 = ml_dtypes.bfloat16
    # zero-pad H and W by 1 on the host: border handling costs nothing here
    xp = np.pad(np.asarray(x, dtype=np.float32),
                ((0, 0), (0, 0), (1, 1), (1, 1)))
    xp = np.ascontiguousarray(xp).astype(bf16)
    # weight OIHW -> [ci, (kh kw co)] so each lhsT tile is a contiguous slice
    w_host = np.ascontiguousarray(
        np.asarray(weight, dtype=np.float32).transpose(1, 2, 3, 0).reshape(C_IN, 9 * C_OUT)
    ).astype(bf16)
    # bias[co] -> [co % 128, co // 128]
    b_host = np.ascontiguousarray(
        np.asarray(bias, dtype=np.float32).reshape(N_CT, 128).T)
    return xp, w_host, b_host


def kernel(x, weight, bias, _trace=False, _repeat=1):
    xp, w_host, b_host = _host_prep(x, weight, bias)
    nc = build_nc(repeat=_repeat)
    in_maps = [
        {"x": xp[i * N_PER_CORE:(i + 1) * N_PER_CORE], "w": w_host, "b": b_host}
        for i in range(N_CORES)
    ]
    res = run_bass_kernel_spmd(nc, in_maps, core_ids=list(range(N_CORES)), trace=_trace)
    out = np.concatenate(
        [res.results[i]["out"].astype(np.float32).reshape(N_PER_CORE, C_OUT, H, W)
         for i in range(N_CORES)],
        axis=0,
    )
    if _trace:
        return out, res
    return out
